# revision 21
# baseline (speedup 1.0000x reference)
"""CapsuleTransformConv on 8 Trainium2 NeuronCores (bf16/fp16 pipeline).

Problem:  x [4,16,16,32,16] f32, matrix [288,16,512] f32.
          im2col (K=3, VALID) -> tile [4,14,14,288,16]
          votes  = einsum('bhwna,nac->bhwnc', tile, matrix)
          out    = votes.reshape(4,14,14,288,32,16)

Sharding: tensor-parallel over the filter*atom output axis (512 -> 64 per
core).  Every core reads the full x and its 64-wide weight slice; writes
its 784 x 288 x 64 output slice (fp16, ~29 MB -- the dominant traffic).

Design (v4), driven by trace analysis of earlier revisions:
  - Output is fp16 (harness gate is rel_err < 2e-2; measured ~3e-3 total
    with bf16 matmul inputs).  Host converts back to f32 (free).
  - x and the weights are cast/packed ON HOST: x uploads as bf16 (1 MB),
    weights upload as the block-diagonal wpack[9, 128, 2048] bf16
    (wpack[kk][(gc,a), oct*512+gc*64+f] = matrix[kk*32+oct*8+gc, a, f]).
    No on-chip weight build at all; the 9 x 512 KB wpack loads ride the
    otherwise-idle GPSIMD SWDGE ring.
  - Weights-stationary matmuls: stationary = wpack chunk [K=128, M=128
    f-cols], moving = tap positions.  Every output is M=128 wide and the
    output is f-major o[kk, f=2048, pos=784] (host untangles).  PSUM
    rules (out <= 512 f32/partition, bank-aligned) make each chunk two
    matmuls, N=512 + N=272, into one [128,784] 2-bank PSUM tile.
  - One whole-chunk PSUM->SBUF fp16 cast per chunk (fixed per-op cost
    ~230 ns makes split copies wasteful), alternating DVE / ACT;
    two chunks stage into a [128, 2*784] fp16 tile and leave in one
    401 KB contiguous DMA issued from the Sync ring (HWDGE), keeping
    both copy engines free of DMA-issue work.
  - Tap compaction (im2col gather): tap 0 per-batch on DVE/ACT right
    after each batch's transposes (first matmul ~10 us in); taps >= 1
    on GPSIMD (octs 0-1) / DVE (oct 2) / ACT (oct 3), prefetched a tap
    ahead via double-buffered tap tiles.
"""

import numpy as np

B, H, W, C, A = 4, 16, 16, 32, 16
KS = 3
OH = OW = 14
NCAP = KS * KS * C          # 288 capsules
FTOT = 512                  # filter*atom
NCORES = 8
FPC = FTOT // NCORES        # 64 output features per core
POS = B * OH * OW           # 784 output positions

_NC_CACHE = {}


def _build_nc():
    import concourse.bass as bass  # noqa: F401
    import concourse.mybir as mybir
    import concourse.tile as tile
    from concourse import bacc, masks

    f16 = mybir.dt.float16
    f32 = mybir.dt.float32
    bf16 = mybir.dt.bfloat16

    nc = bacc.Bacc(None, target_bir_lowering=False)
    x_d = nc.declare_dram_parameter("x", [B * H * W, C * A], bf16,
                                    isOutput=False)
    w_d = nc.declare_dram_parameter("wpack", [128, KS * KS * 256], bf16,
                                    isOutput=False)
    # f-major output: o[kk, f(oct*512+gc*64+f64), pos].
    o_d = nc.declare_dram_parameter("out", [KS * KS, 2048, POS], f16,
                                    isOutput=True)

    # pair-of-chunks view [9, 8, 128, 2, 784]: one DMA per 2 chunks.
    ov = o_d.rearrange("k (g h p) q -> k g p h q", p=128, h=2)

    with tile.TileContext(nc) as tc:
        with (
            tc.tile_pool(name="const", bufs=1) as constp,
            tc.tile_pool(name="big", bufs=1) as bigp,
            tc.tile_pool(name="stage", bufs=4) as stagep,
            tc.tile_pool(name="tapp", bufs=2) as tapp,
            tc.tile_pool(name="psumtr", bufs=2, space="PSUM") as psumtr,
            tc.tile_pool(name="psuma", bufs=3, space="PSUM") as psuma,
            tc.tile_pool(name="psumb", bufs=3, space="PSUM") as psumb,
        ):
            ident = constp.tile([128, 128], bf16, tag="ident")
            masks.make_identity(nc, ident[:])

            # ---- weights: compact 590 KB in DRAM; zeros come from one
            # GPSIMD memset of wpall and 8 paint DMAs (one per gc band,
            # covering all 9 taps) scatter the diagonal blocks ----
            wpall = bigp.tile([128, 9 * 2048], bf16, tag="wpall",
                              name="wpall")
            nc.gpsimd.memset(wpall[:], 0.0)
            wpv = wpall[:].rearrange("p (k o v) -> p k o v", k=9, o=4)
            wsv = w_d.rearrange("p (k o f) -> p k o f", k=9, o=4)
            for gc in range(8):
                eng = nc.sync if gc % 2 == 0 else nc.scalar
                eng.dma_start(
                    wpv[gc * 16:(gc + 1) * 16, :, :,
                        gc * FPC:(gc + 1) * FPC],
                    wsv[gc * 16:(gc + 1) * 16],
                )

            # ---- x (bf16): four [128, 2*512] tiles; tile t = batch t ----
            xsrc = x_d.rearrange("(t s p) c -> t p s c", t=4, p=128)
            x16s = [
                bigp.tile([128, 2 * 512], bf16, tag=f"x16_{t}", name=f"x16_{t}")
                for t in range(4)
            ]
            for t in range(4):
                eng = nc.sync if t % 2 == 0 else nc.scalar
                eng.dma_start(
                    x16s[t][:].rearrange("p (s c) -> p s c", s=2), xsrc[t]
                )

            # ---- PE-transpose into per-octet xt[oct][(dc,a), (b,h,w)] ----
            xts = [
                bigp.tile([128, 1024], bf16, tag=f"xt{o}", name=f"xt{o}")
                for o in range(4)
            ]
            xtvs = [
                t[:].rearrange("p (b h w) -> p b h w", b=B, h=H) for t in xts
            ]
            # row-compacted tap: tapI[ki][(dc,a), oct*896 + (b,i,w)]
            # keeps full W=16 rows; the matmul streams the kj-shifted
            # window and the PSUM->SBUF copy drops the 2 invalid w cols.
            RL = OH * W  # 224 per batch
            tap0 = tapp.tile([128, 4 * 4 * RL], bf16, tag="tap")
            t0v = [
                tap0[:, o * 4 * RL:(o + 1) * 4 * RL].rearrange(
                    "p (b i w) -> p b i w", b=B, i=OH
                )
                for o in range(4)
            ]
            for t in range(4):
                for s in (2 * t, 2 * t + 1):
                    for oct in range(4):
                        tr = psumtr.tile([128, 128], bf16, tag="tr")
                        nc.tensor.transpose(
                            tr[:],
                            x16s[t][
                                :, (s % 2) * 512 + oct * 128:
                                (s % 2) * 512 + (oct + 1) * 128
                            ],
                            ident[:],
                        )
                        dst = xts[oct][:, s * 128:(s + 1) * 128]
                        if (s + oct) % 2 == 0:
                            nc.vector.tensor_copy(dst, tr[:])
                        else:
                            nc.scalar.copy(dst, tr[:])
                # batch t of tap 0 compacts as soon as its transposes land
                for oct in range(4):
                    src = xtvs[oct][:, t:t + 1, 0:OH, :]
                    if (t + oct) % 2 == 0:
                        nc.vector.tensor_copy(t0v[oct][:, t:t + 1], src)
                    else:
                        nc.scalar.copy(t0v[oct][:, t:t + 1], src)

            # ---- main loop: 9 taps x 4 octs x 4 chunks ----
            # tapI[ki] built once per ki (3 contiguous-row builds);
            # matmuls N=512|384-kj cover the kj-shifted window; one
            # strided whole-chunk PSUM->SBUF fp16 cast (keep 14 of 16 w)
            # per chunk, alternating DVE | ACT.
            tapi = tap0
            it = 0
            for kk in range(9):
                ki, kj = kk // 3, kk % 3
                if kj == 0 and ki > 0:
                    tapi = tapp.tile([128, 4 * 4 * RL], bf16, tag="tap")
                    for oct in range(4):
                        dst = tapi[:, oct * 4 * RL:(oct + 1) * 4 * RL]
                        nc.gpsimd.tensor_copy(
                            dst.rearrange("p (b r) -> p b r", b=B),
                            xtvs[oct][:, :, ki: ki + OH, :].rearrange(
                                "p b i w -> p b (i w)"
                            ),
                        )
                n2 = 384 - kj
                for oct in range(4):
                    for c2 in range(2):
                        st = stagep.tile([128, 2 * POS], f16, tag="st")
                        for h2 in range(2):
                            ch = c2 * 2 + h2
                            wchunk = wpall[
                                :, kk * 2048 + oct * 512 + ch * 128:
                                kk * 2048 + oct * 512 + (ch + 1) * 128
                            ]
                            # two single-bank PSUM tiles per chunk
                            # (N=512 | 384-kj); each half's 14-of-16 w
                            # gather cast starts as soon as its matmul
                            # lands, one half per engine.
                            base = oct * 4 * RL + kj
                            dstv = st[:, h2 * POS:(h2 + 1) * POS].rearrange(
                                "p (r j) -> p r j", j=OW
                            )
                            psa = psuma.tile([128, 512], f32, tag="a")
                            nc.tensor.matmul(
                                psa[:], wchunk,
                                tapi[:, base: base + 512],
                                start=True, stop=True,
                            )
                            sa = psa[:].rearrange(
                                "p (r w) -> p r w", w=W)[:, :, 0:OW]
                            psb = psumb.tile([128, 384], f32, tag="b")
                            nc.tensor.matmul(
                                psb[:, 0:n2], wchunk,
                                tapi[:, base + 512: base + 512 + n2],
                                start=True, stop=True,
                            )
                            sb = psb[:].rearrange(
                                "p (r w) -> p r w", w=W)[:, :, 0:OW]
                            if it % 2 == 0:
                                nc.vector.tensor_copy(dstv[:, 0:32], sa)
                                nc.scalar.copy(dstv[:, 32:56], sb)
                            else:
                                nc.scalar.copy(dstv[:, 0:32], sa)
                                nc.vector.tensor_copy(dstv[:, 32:56], sb)
                            it += 1
                        nc.sync.dma_start(
                            ov[kk, oct * 2 + c2],
                            st[:].rearrange("p (h q) -> p h q", h=2),
                        )

    nc.compile()
    return nc


def _get_nc():
    if "nc" not in _NC_CACHE:
        _NC_CACHE["nc"] = _build_nc()
    return _NC_CACHE["nc"]


def _pack_weights(matrix):
    """matrix [288,16,512] f32 -> per-core compact wc [8][128, 9*256]
    bf16.  wc[c][gc*16+a, kk*256+oct*64+f] = matrix[kk*32+oct*8+gc, a,
    c*64+f]; the kernel expands the block-diagonal on-chip."""
    import ml_dtypes
    m = matrix.reshape(KS * KS, 4, 8, A, NCORES, FPC)  # [kk,oct,gc,a,core,f]
    out = m.transpose(4, 2, 3, 0, 1, 5).reshape(NCORES, 128, KS * KS * 256)
    return np.ascontiguousarray(out).astype(ml_dtypes.bfloat16)


def _core_inputs(x, matrix):
    import ml_dtypes
    xb = np.ascontiguousarray(
        np.asarray(x, dtype=np.float32).reshape(B * H * W, C * A)
    ).astype(ml_dtypes.bfloat16)
    wp = _pack_weights(np.asarray(matrix, dtype=np.float32))
    return [
        {"x": xb, "wpack": np.ascontiguousarray(wp[c])}
        for c in range(NCORES)
    ]


def _unscramble(parts):
    """parts: [8][9, 2048, 784] fp16 -> [4,14,14,288,32,16] f32."""
    arr = np.stack(parts)                              # [core,kk,col,pos]
    arr = arr.reshape(NCORES, KS * KS, 4, 8, FPC, POS)
    arr = arr.transpose(5, 1, 2, 3, 0, 4)              # [pos,kk,oct,gc,core,f]
    full = arr.reshape(POS, NCAP, FTOT).astype(np.float32)
    return np.ascontiguousarray(
        full.reshape(B, OH, OW, NCAP, 32, 16)
    )


def kernel(x, matrix):
    from concourse.bass_utils import run_bass_kernel_spmd

    nc = _get_nc()
    in_maps = _core_inputs(x, matrix)
    r = run_bass_kernel_spmd(nc, in_maps, list(range(NCORES)))
    return _unscramble([r.results[c]["out"] for c in range(NCORES)])
